# revision 1
# baseline (speedup 1.0000x reference)
"""Trainium2 Bass kernel for ConvCapsuleLayer (SegCaps conv-capsule + dynamic routing).

Reference computation (per full input):
  x [B=4,H=128,W=128,IC=4,IA=16] -> conv 5x5 SAME over each of IC planes
  (IA in-channels -> NC*NA=128 out-channels) -> votes [IC,B,h,w,NC,NA]
  -> 3 iterations of dynamic routing (softmax over NC, squash over NA)
  -> activation [B,h,w,NC,NA]

Sharding: 8 cores = batch(4) x row-halves(2). Each core computes rows
[r0, r0+64) of one batch image: conv for all IC + full routing (routing is
per-pixel, so fully shard-local). Output gathered by concatenation.

Per-core layout: output channels (nc,na)=128 on SBUF partitions, pixels on
the free axis, processed in "quads" of 4 image rows (512 pixels); 4 quads
form a "super" so the small softmax/squash tensors pack 4 quads into one
[32,512]/[128,512] instruction. Full fp32 (routing amplifies input noise
~10x, so bf16 anywhere costs percent-level output error).

Conv: K=(dy,ia)=80 contraction, 5 matmuls (one per dx) accumulating in PSUM;
the input is staged host-side as 5 row-shifted copies (one per dy) so each
output row's receptive field is a contiguous AP slice. A 5th "summed over
IC" image gives routing iteration 1's uniform-route preactivation directly.
"""

import os

import numpy as np
import ml_dtypes

import concourse.bass as bass
import concourse.bacc as bacc
import concourse.mybir as mybir
import concourse.tile as tile
from concourse.bass_utils import run_bass_kernel_spmd

BF16 = mybir.dt.bfloat16
F32 = mybir.dt.float32
F32R = mybir.dt.float32r
AF = mybir.ActivationFunctionType
OP = mybir.AluOpType

IC, IA, NC, NA, KS = 4, 16, 8, 16, 5
ROWS = 64            # rows per core
WID = 128            # image width
PIX = ROWS * WID     # 8192 pixels per core
NSUPER = 4           # supers per core, 4 quads (16 rows) each

_NC_CACHE = {}
LAST_EXEC_NS = None


def _consts():
    """Stationary 0/1 matrices for partition-group reduce / broadcast."""
    # S16[j]: [128,32] reduce sq over na (groups of 16) into rows 8j..8j+8
    s16 = np.zeros((4, 128, 32), np.float32)
    for j in range(4):
        for m in range(32):
            if m // 8 == j:
                nc_i = m % 8
                s16[j, nc_i * 16:(nc_i + 1) * 16, m] = 1.0
    # S8x16[j]: [32,128] broadcast factor rows 8j+nc -> partitions (nc,na)
    s816 = np.zeros((4, 32, 128), np.float32)
    for j in range(4):
        for m in range(128):
            s816[j, 8 * j + m // 16, m] = 1.0
    # Bic[ic]: [32,128] broadcast route rows (8ic+nc) -> partitions (nc,na)
    bic = np.zeros((4, 32, 128), np.float32)
    for ic in range(4):
        for m in range(128):
            bic[ic, 8 * ic + m // 16, m] = 1.0
    # Sagr[j,ic]: [128,128] reduce G over na into logits rows 32j+8ic..+8
    sagr = np.zeros((16, 128, 128), np.float32)
    for j in range(4):
        for ic in range(4):
            for m in range(128):
                if m // 32 == j and (m % 32) // 8 == ic:
                    nc_i = m % 8
                    sagr[4 * j + ic, nc_i * 16:(nc_i + 1) * 16, m] = 1.0
    # D128: [128,128] block softmax denominator (sum+broadcast within 8-blocks)
    d128 = np.zeros((128, 128), np.float32)
    for k in range(128):
        for m in range(128):
            if k // 8 == m // 8:
                d128[k, m] = 1.0
    ident = np.eye(128, dtype=np.float32)
    return s16, s816, bic, sagr, d128, ident


def _build_module(mode=None):
    nc = bacc.Bacc(None, target_bir_lowering=False)
    x5_d = nc.declare_dram_parameter("x5", [80, IC, ROWS, WID + 4], F32, isOutput=False)
    xs5_d = nc.declare_dram_parameter("xs5", [80, ROWS, WID + 4], F32, isOutput=False)
    wts_d = nc.declare_dram_parameter("wts", [KS, 80, 128], F32, isOutput=False)
    bias_d = nc.declare_dram_parameter("bias", [128, 1], F32, isOutput=False)
    y_d = nc.declare_dram_parameter("y", [PIX, 128], F32, isOutput=True)

    s16_np, s816_np, bic_np, sagr_np, d128_np, ident_np = _consts()
    s16_d = nc.inline_tensor(np.ascontiguousarray(s16_np.transpose(1, 0, 2)), "s16c")
    s816_d = nc.inline_tensor(np.ascontiguousarray(s816_np.transpose(1, 0, 2)), "s816c")
    bic_d = nc.inline_tensor(np.ascontiguousarray(bic_np.transpose(1, 0, 2)), "bicc")
    sagr_d = nc.inline_tensor(np.ascontiguousarray(sagr_np.transpose(1, 0, 2)), "sagrc")
    d128_d = nc.inline_tensor(d128_np, "d128c")
    ident_d = nc.inline_tensor(ident_np, "identc")

    with tile.TileContext(nc) as tc:
        _body(tc, x5_d, xs5_d, wts_d, bias_d, y_d,
              s16_d, s816_d, bic_d, sagr_d, d128_d, ident_d, mode=mode)
    nc.compile()
    return nc


def _body(tc, x5_d, xs5_d, wts_d, bias_d, y_d,
          s16_d, s816_d, bic_d, sagr_d, d128_d, ident_d, mode=None):
    if mode is None:
        mode = {"softmax", "preact", "squash", "agree", "out"}
    nc = tc.nc
    from contextlib import ExitStack
    ctx = ExitStack()
    with ctx:
        consts = ctx.enter_context(tc.tile_pool(name="consts", bufs=1))
        xpool = ctx.enter_context(tc.tile_pool(name="xin", bufs=3))
        vpool = ctx.enter_context(tc.tile_pool(name="votes", bufs=7))
        prepool = ctx.enter_context(tc.tile_pool(name="pre", bufs=5))
        apool = ctx.enter_context(tc.tile_pool(name="act", bufs=5))
        a3pool = ctx.enter_context(tc.tile_pool(name="act3", bufs=2))
        tmppool = ctx.enter_context(tc.tile_pool(name="tmp", bufs=2))
        gpool = ctx.enter_context(tc.tile_pool(name="gtmp", bufs=4))
        spool = ctx.enter_context(tc.tile_pool(name="smax", bufs=2))
        smallpool = ctx.enter_context(tc.tile_pool(name="small", bufs=2))
        ps_conv = ctx.enter_context(tc.tile_pool(name="ps_conv", bufs=3, space="PSUM"))
        ps_rb = ctx.enter_context(tc.tile_pool(name="ps_rb", bufs=2, space="PSUM"))
        ps_logit = ctx.enter_context(tc.tile_pool(name="ps_logit", bufs=1, space="PSUM"))
        ps_misc = ctx.enter_context(tc.tile_pool(name="ps_misc", bufs=2, space="PSUM"))

        # ---- constants into SBUF
        w_sb = consts.tile([80, KS, 128], F32R)
        for dx in range(KS):
            nc.sync.dma_start(out=w_sb[:, dx, :], in_=wts_d[dx].bitcast(F32R))
        bias_sb = consts.tile([128, 1], F32)
        nc.sync.dma_start(out=bias_sb, in_=bias_d[:])
        s16_sb = consts.tile([128, 4, 32], F32R)
        nc.sync.dma_start(out=s16_sb, in_=s16_d[:].bitcast(F32R))
        s816_sb = consts.tile([32, 4, 128], F32R)
        nc.sync.dma_start(out=s816_sb, in_=s816_d[:].bitcast(F32R))
        bic_sb = consts.tile([32, 4, 128], F32R)
        nc.sync.dma_start(out=bic_sb, in_=bic_d[:].bitcast(F32R))
        sagr_sb = consts.tile([128, 16, 128], F32R)
        nc.sync.dma_start(out=sagr_sb, in_=sagr_d[:].bitcast(F32R))
        d128_sb = consts.tile([128, 128], F32R)
        nc.sync.dma_start(out=d128_sb, in_=d128_d[:].bitcast(F32R))
        id_sb = consts.tile([128, 128], F32)
        nc.sync.dma_start(out=id_sb, in_=ident_d[:])

        y_ap = y_d[:]

        def mm(out, lhsT, rhs, **kw):
            nc.tensor.matmul(out, lhsT=lhsT, rhs=rhs, **kw)

        xcs0 = None
        xss0 = None
        for s in range(NSUPER):
            # ---- stream this super's input rows (16) in two 8-row chunks
            if "nodma" in mode and xcs0 is not None:
                xcs, xss = xcs0, xss0
            else:
                xcs = []
                xss = []
                for h in range(4):
                    r = 16 * s + 4 * h
                    xc = xpool.tile([80, IC, 4, WID + 4], F32R, tag="xc")
                    nc.sync.dma_start(out=xc, in_=x5_d[:, :, r:r + 4, :].bitcast(F32R))
                    xcs.append(xc)
                    xsh = xpool.tile([80, 4, WID + 4], F32R, tag="xs")
                    nc.sync.dma_start(out=xsh, in_=xs5_d[:, r:r + 4, :].bitcast(F32R))
                    xss.append(xsh)
                xcs0, xss0 = xcs, xss

            # ---- conv: votes for 4 quads + uniform-route preactivation
            V = []
            P1 = []
            for j in range(4):
                vj = vpool.tile([128, IC, 512], F32, tag="vtile")
                for ic in range(IC):
                    pv = ps_conv.tile([128, 512], F32, tag="pconv")
                    for dx in range(KS):
                        mm(pv[:],
                           w_sb[:, dx, :],
                           xcs[j][:, ic, :, dx:dx + WID],
                           start=(dx == 0), stop=(dx == KS - 1))
                    if ic % 2 == 0:
                        nc.scalar.copy(out=vj[:, ic, :], in_=pv[:])
                    else:
                        nc.vector.tensor_copy(out=vj[:, ic, :], in_=pv[:])
                V.append(vj)
                ps1 = ps_conv.tile([128, 512], F32, tag="pconv")
                for dx in range(KS):
                    mm(ps1[:], w_sb[:, dx, :],
                       xss[j][:, :, dx:dx + WID],
                       start=(dx == 0), stop=(dx == KS - 1))
                P1.append(ps1)

            logits = ps_logit.tile([128, 512], F32, tag="logits")
            if "agree" not in mode:
                nc.vector.memset(logits[:], 0.0)
            A = [None] * 4
            pre = [None] * 4
            for t in (1, 2, 3):
                if t == 1:
                    for j in range(4):
                        pj = prepool.tile([128, 512], F32, tag="pre")
                        nc.scalar.activation(pj, P1[j][:], AF.Identity,
                                             bias=bias_sb, scale=0.125)
                        pre[j] = pj
                elif "softmax" in mode:
                    expt = spool.tile([128, 512], F32R, tag="expt")
                    nc.scalar.activation(expt, logits[:], AF.Exp)
                    den = ps_misc.tile([128, 512], F32, tag="misc")
                    mm(den[:], d128_sb[:], expt[:], start=True, stop=True)
                    lnden = spool.tile([128, 512], F32, tag="lnden")
                    nc.scalar.activation(lnden, den[:], AF.Ln)
                    rden = spool.tile([128, 512], F32, tag="rden")
                    nc.scalar.activation(rden, lnden, AF.Exp, scale=-1.0)
                    routes = []
                    for j in range(4):
                        rj = spool.tile([32, 512], F32R, tag="routes")
                        nc.vector.tensor_mul(rj,
                                             expt[32 * j:32 * j + 32, :],
                                             rden[32 * j:32 * j + 32, :])
                        routes.append(rj)
                    for j in range(4):
                        if "preact" not in mode:
                            if pre[j] is None:
                                dummy_pre = prepool.tile([128, 512], F32, tag="pre")
                                nc.vector.memset(dummy_pre, 0.0)
                                pre[j] = dummy_pre
                            continue
                        gs = []
                        for ic in range(IC):
                            rb = ps_rb.tile([128, 512], F32, tag="rb")
                            mm(rb[:], bic_sb[:, ic, :], routes[j][:],
                               start=True, stop=True)
                            g = gpool.tile([128, 512], F32, tag="g")
                            nc.vector.tensor_mul(g, V[j][:, ic, :], rb[:])
                            gs.append(g)
                        a01 = tmppool.tile([128, 512], F32, tag="s01")
                        nc.vector.tensor_add(a01, gs[0], gs[1])
                        a23 = tmppool.tile([128, 512], F32, tag="s23")
                        nc.vector.tensor_add(a23, gs[2], gs[3])
                        acc = tmppool.tile([128, 512], F32, tag="vsum")
                        nc.vector.tensor_add(acc, a01, a23)
                        pj = prepool.tile([128, 512], F32, tag="pre")
                        nc.vector.tensor_scalar_add(out=pj, in0=acc, scalar1=bias_sb)
                        pre[j] = pj

                if "squash" not in mode:
                    for j in range(4):
                        if t < 3 and A[j] is None:
                            dummy_a = apool.tile([128, 512], F32, tag="atile")
                            nc.vector.memset(dummy_a, 0.0)
                            A[j] = dummy_a
                    continue
                # squash factor, packed over the 4 quads
                n2 = ps_misc.tile([32, 512], F32, tag="misc")
                for j in range(4):
                    sq = tmppool.tile([128, 512], F32R, tag="sq")
                    nc.gpsimd.tensor_mul(sq, pre[j], pre[j])
                    mm(n2[:], s16_sb[:, j, :], sq[:], start=(j == 0), stop=(j == 3))
                ln1 = smallpool.tile([32, 512], F32, tag="ln1")
                nc.scalar.activation(ln1, n2[:], AF.Ln, bias=1.0)
                lnn = smallpool.tile([32, 512], F32, tag="lnn")
                nc.scalar.activation(lnn, n2[:], AF.Ln)
                tfac = smallpool.tile([32, 512], F32, tag="tfac")
                nc.vector.scalar_tensor_tensor(
                    out=tfac, in0=lnn, scalar=0.5, in1=ln1,
                    op0=OP.mult, op1=OP.subtract)
                fac = smallpool.tile([32, 512], F32R, tag="fac")
                nc.scalar.activation(fac, tfac, AF.Exp)

                for j in range(4):
                    fb = ps_rb.tile([128, 512], F32, tag="rb")
                    mm(fb[:], s816_sb[:, j, :], fac[:], start=True, stop=True)
                    if t < 3:
                        aj = apool.tile([128, 512], F32, tag="atile")
                        nc.vector.tensor_mul(aj, pre[j], fb[:])
                        A[j] = aj
                    elif "out" not in mode:
                        pass
                    else:
                        a3 = a3pool.tile([128, 512], F32, tag="a3")
                        nc.vector.tensor_mul(a3, pre[j], fb[:])
                        tt = ps_misc.tile([128, 4, 128], F32, tag="misc")
                        for u in range(4):
                            nc.tensor.transpose(tt[:, u, :],
                                                a3[:, 128 * u:128 * (u + 1)],
                                                id_sb[:])
                        tsb = a3pool.tile([128, 4, 128], F32, tag="tsb")
                        nc.scalar.copy(out=tsb[:], in_=tt[:])
                        q = 4 * s + j
                        for u in range(4):
                            base = q * 512 + u * 128
                            nc.sync.dma_start(out=y_ap[base:base + 128, :],
                                              in_=tsb[:, u, :])

                if t < 3 and "agree" in mode:
                    for j in range(4):
                        for ic in range(IC):
                            g = gpool.tile([128, 512], F32R, tag="ag")
                            nc.gpsimd.tensor_mul(g, V[j][:, ic, :], A[j])
                            nc.tensor.matmul(
                                logits[:],
                                lhsT=sagr_sb[:, 4 * j + ic, :],
                                rhs=g[:],
                                start=(t == 1 and j == 0 and ic == 0),
                                stop=(t == 2 and j == 3 and ic == 3),
                                skip_group_check=True)


def _prep_core_inputs(x, W, b, core):
    bb, half = core // 2, core % 2
    r0 = 64 * half
    xb = x[bb]  # [H, W, IC, IA]
    xpad = np.zeros((IC, ROWS + 4, WID + 4, IA), np.float32)
    lo, hi = max(0, r0 - 2), min(128, r0 + 66)
    xpad[:, lo - (r0 - 2):hi - (r0 - 2), 2:WID + 2, :] = \
        xb[lo:hi].transpose(2, 0, 1, 3)
    x5 = np.empty((80, IC, ROWS, WID + 4), np.float32)
    for dy in range(KS):
        x5[dy * 16:(dy + 1) * 16] = xpad[:, dy:dy + ROWS, :, :].transpose(3, 0, 1, 2)
    return {
        "x5": x5,
        "xs5": np.ascontiguousarray(x5.sum(axis=1)),
        "wts": np.ascontiguousarray(W.transpose(1, 0, 2, 3).reshape(KS, 80, 128)
                                    ).astype(np.float32),
        "bias": np.ascontiguousarray(b.reshape(128, 1)).astype(np.float32),
    }


def kernel(x, W, b):
    global LAST_EXEC_NS
    x = np.asarray(x, np.float32)
    W = np.asarray(W, np.float32)
    b = np.asarray(b, np.float32)

    if "mod" not in _NC_CACHE:
        _NC_CACHE["mod"] = _build_module()
    nc = _NC_CACHE["mod"]

    in_maps = [_prep_core_inputs(x, W, b, c) for c in range(8)]
    trace = os.environ.get("BASS_KERNEL_TRACE", "0") == "1"
    try:
        res = run_bass_kernel_spmd(nc, in_maps, core_ids=list(range(8)), trace=trace)
    except ModuleNotFoundError:
        res = run_bass_kernel_spmd(nc, in_maps, core_ids=list(range(8)), trace=False)
    LAST_EXEC_NS = res.exec_time_ns if res.exec_time_ns else res.mean_exec_time_ns
    if LAST_EXEC_NS is None:
        # No NTFF profiling hook under this axon client; fall back to the
        # instruction-level device-occupancy model (same cost tables CoreSim
        # uses), which is the best available per-core duration estimate.
        if "model_ns" not in _NC_CACHE:
            try:
                from concourse.timeline_sim import TimelineSim
                _NC_CACHE["model_ns"] = int(TimelineSim(nc, trace=False).simulate())
            except Exception:
                _NC_CACHE["model_ns"] = None
        LAST_EXEC_NS = _NC_CACHE["model_ns"]

    out = np.empty((4, 128, 128, NC, NA), np.float32)
    for c in range(8):
        bb, half = c // 2, c % 2
        r0 = 64 * half
        out[bb, r0:r0 + 64] = res.results[c]["y"].reshape(ROWS, WID, NC, NA)
    return out



# revision 6
# speedup vs baseline: 1.0979x; 1.0979x over previous
"""Trainium2 Bass kernel for ConvCapsuleLayer (SegCaps conv-capsule + dynamic routing).

Reference computation (per full input):
  x [B=4,H=128,W=128,IC=4,IA=16] -> conv 5x5 SAME over each of IC planes
  (IA in-channels -> NC*NA=128 out-channels) -> votes [IC,B,h,w,NC,NA]
  -> 3 iterations of dynamic routing (softmax over NC, squash over NA)
  -> activation [B,h,w,NC,NA]

Sharding: 8 cores = batch(4) x row-halves(2). Each core computes rows
[r0, r0+64) of one batch image: conv for all IC + full routing (routing is
per-pixel, so fully shard-local). Output gathered by concatenation.

Per-core layout: output channels (nc,na)=128 on SBUF partitions, pixels on
the free axis, processed in "quads" of 4 image rows (512 pixels); 4 quads
form a "super" so the small softmax/squash tensors pack 4 quads into one
[32,512]/[128,512] instruction. Full fp32 (routing amplifies input noise
~10x, so bf16 anywhere costs percent-level output error).

Conv: K=(dy,ia)=80 contraction, 5 matmuls (one per dx) accumulating in PSUM;
the input is staged host-side as 5 row-shifted copies (one per dy) so each
output row's receptive field is a contiguous AP slice. A 5th "summed over
IC" image gives routing iteration 1's uniform-route preactivation directly.
"""

import os

import numpy as np
import ml_dtypes

import concourse.bass as bass
import concourse.bacc as bacc
import concourse.mybir as mybir
import concourse.tile as tile
from concourse.bass_utils import run_bass_kernel_spmd

BF16 = mybir.dt.bfloat16
F32 = mybir.dt.float32
F32R = mybir.dt.float32r
AF = mybir.ActivationFunctionType
OP = mybir.AluOpType

IC, IA, NC, NA, KS = 4, 16, 8, 16, 5
ROWS = 64            # rows per core
WID = 128            # image width
PIX = ROWS * WID     # 8192 pixels per core
NSUPER = 4           # supers per core, 4 quads (16 rows) each

_NC_CACHE = {}
LAST_EXEC_NS = None


def _consts():
    """Stationary 0/1 matrices for partition-group reduce / broadcast."""
    # S16[j]: [128,32] reduce sq over na (groups of 16) into rows 8j..8j+8
    s16 = np.zeros((4, 128, 32), np.float32)
    for j in range(4):
        for m in range(32):
            if m // 8 == j:
                nc_i = m % 8
                s16[j, nc_i * 16:(nc_i + 1) * 16, m] = 1.0
    # S8x16[j]: [32,128] broadcast factor rows 8j+nc -> partitions (nc,na)
    s816 = np.zeros((4, 32, 128), np.float32)
    for j in range(4):
        for m in range(128):
            s816[j, 8 * j + m // 16, m] = 1.0
    # Bic[ic]: [32,128] broadcast route rows (8ic+nc) -> partitions (nc,na)
    bic = np.zeros((4, 32, 128), np.float32)
    for ic in range(4):
        for m in range(128):
            bic[ic, 8 * ic + m // 16, m] = 1.0
    # Sagr[j,ic]: [128,128] reduce G over na into logits rows 32j+8ic..+8
    sagr = np.zeros((16, 128, 128), np.float32)
    for j in range(4):
        for ic in range(4):
            for m in range(128):
                if m // 32 == j and (m % 32) // 8 == ic:
                    nc_i = m % 8
                    sagr[4 * j + ic, nc_i * 16:(nc_i + 1) * 16, m] = 1.0
    # D128: [128,128] block softmax denominator (sum+broadcast within 8-blocks)
    d128 = np.zeros((128, 128), np.float32)
    for k in range(128):
        for m in range(128):
            if k // 8 == m // 8:
                d128[k, m] = 1.0
    ident = np.eye(128, dtype=np.float32)
    return s16, s816, bic, sagr, d128, ident


def _build_module(mode=None):
    nc = bacc.Bacc(None, target_bir_lowering=False)
    x5_d = nc.declare_dram_parameter("x5", [80, IC, ROWS, WID + 4], F32, isOutput=False)
    xs5_d = nc.declare_dram_parameter("xs5", [80, ROWS, WID + 4], F32, isOutput=False)
    wts_d = nc.declare_dram_parameter("wts", [KS, 80, 128], F32, isOutput=False)
    bias_d = nc.declare_dram_parameter("bias", [128, 1], F32, isOutput=False)
    y_d = nc.declare_dram_parameter("y", [128, PIX], F32, isOutput=True)

    s16_np, s816_np, bic_np, sagr_np, d128_np, ident_np = _consts()
    s16_d = nc.inline_tensor(np.ascontiguousarray(s16_np.transpose(1, 0, 2)), "s16c")
    s816_d = nc.inline_tensor(np.ascontiguousarray(s816_np.transpose(1, 0, 2)), "s816c")
    bic_d = nc.inline_tensor(np.ascontiguousarray(bic_np.transpose(1, 0, 2)), "bicc")
    sagr_d = nc.inline_tensor(np.ascontiguousarray(sagr_np.transpose(1, 0, 2)), "sagrc")
    d128_d = nc.inline_tensor(d128_np, "d128c")
    ident_d = nc.inline_tensor(ident_np, "identc")

    with tile.TileContext(nc) as tc:
        _body(tc, x5_d, xs5_d, wts_d, bias_d, y_d,
              s16_d, s816_d, bic_d, sagr_d, d128_d, ident_d, mode=mode)
    _preload_act_table(nc)
    nc.compile()
    return nc


def _preload_act_table(nc):
    """Pre-place one LoadActFuncSet for the table containing ALL activation
    functions this kernel uses (exp, ln, identity, copy, square). Without it,
    the compile pass picks the first table per function (exp-only / ln-only)
    and thrashes 41 table reloads (1.3us each) onto the critical path."""
    set_id = 6  # natural_log_exp_and_others in act_info.json
    try:
        from concourse.hw_specs import get_activation_tables
        need = {AF.Exp, AF.Ln, AF.Identity, AF.Copy, AF.Square}
        for i, funcs in enumerate(get_activation_tables(nc.m.arch).values()):
            if need <= funcs:
                set_id = i
                break
    except Exception:
        pass
    entry = nc.main_func.blocks[0]
    atl = mybir.InstLoadActFuncSet(name=nc.get_next_instruction_name(),
                                   ins=[], outs=[])
    atl.engine = mybir.EngineType.Activation
    atl.act_func_set_id = set_id
    nc.register_instruction(atl)
    idx = entry.instructions.index(nc.gpsimd.preamble_end) + 1
    entry.instructions.insert(idx, atl)


def _body(tc, x5_d, xs5_d, wts_d, bias_d, y_d,
          s16_d, s816_d, bic_d, sagr_d, d128_d, ident_d, mode=None):
    if mode is None:
        mode = {"softmax", "preact", "squash", "agree", "out"}
    nc = tc.nc
    from contextlib import ExitStack
    ctx = ExitStack()
    with ctx:
        consts = ctx.enter_context(tc.tile_pool(name="consts", bufs=1))
        xpool = ctx.enter_context(tc.tile_pool(name="xin", bufs=3))
        vpool = ctx.enter_context(tc.tile_pool(name="votes", bufs=7))
        prepool = ctx.enter_context(tc.tile_pool(name="pre", bufs=5))
        apool = ctx.enter_context(tc.tile_pool(name="act", bufs=5))
        a3pool = ctx.enter_context(tc.tile_pool(name="act3", bufs=2))
        tmppool = ctx.enter_context(tc.tile_pool(name="tmp", bufs=2))
        gpool = ctx.enter_context(tc.tile_pool(name="gtmp", bufs=4))
        spool = ctx.enter_context(tc.tile_pool(name="smax", bufs=2))
        smallpool = ctx.enter_context(tc.tile_pool(name="small", bufs=2))
        ps_conv = ctx.enter_context(tc.tile_pool(name="ps_conv", bufs=3, space="PSUM"))
        ps_rb = ctx.enter_context(tc.tile_pool(name="ps_rb", bufs=2, space="PSUM"))
        ps_logit = ctx.enter_context(tc.tile_pool(name="ps_logit", bufs=1, space="PSUM"))
        ps_misc = ctx.enter_context(tc.tile_pool(name="ps_misc", bufs=2, space="PSUM"))

        # ---- constants into SBUF
        w_sb = consts.tile([80, KS, 128], F32R)
        for dx in range(KS):
            nc.sync.dma_start(out=w_sb[:, dx, :], in_=wts_d[dx].bitcast(F32R))
        bias_sb = consts.tile([128, 1], F32)
        nc.sync.dma_start(out=bias_sb, in_=bias_d[:])
        s16_sb = consts.tile([128, 4, 32], F32R)
        nc.sync.dma_start(out=s16_sb, in_=s16_d[:].bitcast(F32R))
        s816_sb = consts.tile([32, 4, 128], F32R)
        nc.sync.dma_start(out=s816_sb, in_=s816_d[:].bitcast(F32R))
        bic_sb = consts.tile([32, 4, 128], F32R)
        nc.sync.dma_start(out=bic_sb, in_=bic_d[:].bitcast(F32R))
        sagr_sb = consts.tile([128, 16, 128], F32R)
        nc.sync.dma_start(out=sagr_sb, in_=sagr_d[:].bitcast(F32R))
        d128_sb = consts.tile([128, 128], F32R)
        nc.sync.dma_start(out=d128_sb, in_=d128_d[:].bitcast(F32R))
        id_sb = consts.tile([128, 128], F32)
        nc.sync.dma_start(out=id_sb, in_=ident_d[:])

        y_ap = y_d[:]

        def mm(out, lhsT, rhs, **kw):
            nc.tensor.matmul(out, lhsT=lhsT, rhs=rhs, **kw)

        xcs0 = None
        xss0 = None
        for s in range(NSUPER):
            # ---- stream this super's input rows (16) in two 8-row chunks
            if "nodma" in mode and xcs0 is not None:
                xcs, xss = xcs0, xss0
            else:
                xcs = []
                xss = []
                for h in range(4):
                    r = 16 * s + 4 * h
                    xc = xpool.tile([80, IC, 4, WID + 4], F32R, tag="xc")
                    nc.sync.dma_start(out=xc, in_=x5_d[:, :, r:r + 4, :].bitcast(F32R))
                    xcs.append(xc)
                    xsh = xpool.tile([80, 4, WID + 4], F32R, tag="xs")
                    nc.sync.dma_start(out=xsh, in_=xs5_d[:, r:r + 4, :].bitcast(F32R))
                    xss.append(xsh)
                xcs0, xss0 = xcs, xss

            # ---- conv: votes for 4 quads + uniform-route preactivation
            V = []
            P1 = []
            for j in range(4):
                vj = vpool.tile([128, IC, 512], F32, tag="vtile")
                for ic in range(IC):
                    pv = ps_conv.tile([128, 512], F32, tag="pconv")
                    for dx in range(KS):
                        mm(pv[:],
                           w_sb[:, dx, :],
                           xcs[j][:, ic, :, dx:dx + WID],
                           start=(dx == 0), stop=(dx == KS - 1))
                    if ic % 2 == 0:
                        nc.scalar.copy(out=vj[:, ic, :], in_=pv[:])
                    else:
                        nc.vector.tensor_copy(out=vj[:, ic, :], in_=pv[:])
                V.append(vj)
                ps1 = ps_conv.tile([128, 512], F32, tag="pconv")
                for dx in range(KS):
                    mm(ps1[:], w_sb[:, dx, :],
                       xss[j][:, :, dx:dx + WID],
                       start=(dx == 0), stop=(dx == KS - 1))
                P1.append(ps1)

            logits = ps_logit.tile([128, 512], F32, tag="logits")
            if "agree" not in mode:
                nc.vector.memset(logits[:], 0.0)
            A = [None] * 4
            pre = [None] * 4
            for t in (1, 2, 3):
                if t == 1:
                    for j in range(4):
                        pj = prepool.tile([128, 512], F32, tag="pre")
                        nc.scalar.activation(pj, P1[j][:], AF.Identity,
                                             bias=bias_sb, scale=0.125)
                        pre[j] = pj
                elif "softmax" in mode:
                    expt = spool.tile([128, 512], F32R, tag="expt")
                    nc.scalar.activation(expt, logits[:], AF.Exp)
                    den = ps_misc.tile([128, 512], F32, tag="misc")
                    mm(den[:], d128_sb[:], expt[:], start=True, stop=True)
                    lnden = spool.tile([128, 512], F32, tag="lnden")
                    nc.scalar.activation(lnden, den[:], AF.Ln)
                    rden = spool.tile([128, 512], F32, tag="rden")
                    nc.scalar.activation(rden, lnden, AF.Exp, scale=-1.0)
                    routes = []
                    for j in range(4):
                        rj = spool.tile([32, 512], F32R, tag="routes")
                        nc.vector.tensor_mul(rj,
                                             expt[32 * j:32 * j + 32, :],
                                             rden[32 * j:32 * j + 32, :])
                        routes.append(rj)
                    for j in range(4):
                        if "preact" not in mode:
                            if pre[j] is None:
                                dummy_pre = prepool.tile([128, 512], F32, tag="pre")
                                nc.vector.memset(dummy_pre, 0.0)
                                pre[j] = dummy_pre
                            continue
                        gs = []
                        for ic in range(IC):
                            rb = ps_rb.tile([128, 512], F32, tag="rb")
                            mm(rb[:], bic_sb[:, ic, :], routes[j][:],
                               start=True, stop=True)
                            g = gpool.tile([128, 512], F32, tag="g")
                            nc.vector.tensor_mul(g, V[j][:, ic, :], rb[:])
                            gs.append(g)
                        a01 = tmppool.tile([128, 512], F32, tag="s01")
                        nc.vector.tensor_add(a01, gs[0], gs[1])
                        a23 = tmppool.tile([128, 512], F32, tag="s23")
                        nc.vector.tensor_add(a23, gs[2], gs[3])
                        acc = tmppool.tile([128, 512], F32, tag="vsum")
                        nc.vector.tensor_add(acc, a01, a23)
                        pj = prepool.tile([128, 512], F32, tag="pre")
                        nc.vector.tensor_scalar_add(out=pj, in0=acc, scalar1=bias_sb)
                        pre[j] = pj

                if "squash" not in mode:
                    for j in range(4):
                        if t < 3 and A[j] is None:
                            dummy_a = apool.tile([128, 512], F32, tag="atile")
                            nc.vector.memset(dummy_a, 0.0)
                            A[j] = dummy_a
                    continue
                # squash factor, packed over the 4 quads
                n2 = ps_misc.tile([32, 512], F32, tag="misc")
                for j in range(4):
                    sq = tmppool.tile([128, 512], F32R, tag="sq")
                    nc.scalar.activation(sq, pre[j], AF.Square)
                    mm(n2[:], s16_sb[:, j, :], sq[:], start=(j == 0), stop=(j == 3))
                ln1 = smallpool.tile([32, 512], F32, tag="ln1")
                nc.scalar.activation(ln1, n2[:], AF.Ln, bias=1.0)
                lnn = smallpool.tile([32, 512], F32, tag="lnn")
                nc.scalar.activation(lnn, n2[:], AF.Ln)
                tfac = smallpool.tile([32, 512], F32, tag="tfac")
                nc.vector.scalar_tensor_tensor(
                    out=tfac, in0=lnn, scalar=0.5, in1=ln1,
                    op0=OP.mult, op1=OP.subtract)
                fac = smallpool.tile([32, 512], F32R, tag="fac")
                nc.scalar.activation(fac, tfac, AF.Exp)

                for j in range(4):
                    fb = ps_rb.tile([128, 512], F32, tag="rb")
                    mm(fb[:], s816_sb[:, j, :], fac[:], start=True, stop=True)
                    if t < 3:
                        aj = apool.tile([128, 512], F32, tag="atile")
                        nc.vector.tensor_mul(aj, pre[j], fb[:])
                        A[j] = aj
                    elif "out" not in mode:
                        pass
                    else:
                        a3 = a3pool.tile([128, 512], F32, tag="a3")
                        nc.vector.tensor_mul(a3, pre[j], fb[:])
                        q = 4 * s + j
                        nc.sync.dma_start(out=y_ap[:, q * 512:(q + 1) * 512],
                                          in_=a3[:])

                if t < 3 and "agree" in mode:
                    for j in range(4):
                        for ic in range(IC):
                            g = gpool.tile([128, 512], F32R, tag="ag")
                            nc.gpsimd.tensor_mul(g, V[j][:, ic, :], A[j])
                            nc.tensor.matmul(
                                logits[:],
                                lhsT=sagr_sb[:, 4 * j + ic, :],
                                rhs=g[:],
                                start=(t == 1 and j == 0 and ic == 0),
                                stop=(t == 2 and j == 3 and ic == 3),
                                skip_group_check=True)


def _prep_core_inputs(x, W, b, core):
    bb, half = core // 2, core % 2
    r0 = 64 * half
    xb = x[bb]  # [H, W, IC, IA]
    xpad = np.zeros((IC, ROWS + 4, WID + 4, IA), np.float32)
    lo, hi = max(0, r0 - 2), min(128, r0 + 66)
    xpad[:, lo - (r0 - 2):hi - (r0 - 2), 2:WID + 2, :] = \
        xb[lo:hi].transpose(2, 0, 1, 3)
    x5 = np.empty((80, IC, ROWS, WID + 4), np.float32)
    for dy in range(KS):
        x5[dy * 16:(dy + 1) * 16] = xpad[:, dy:dy + ROWS, :, :].transpose(3, 0, 1, 2)
    return {
        "x5": x5,
        "xs5": np.ascontiguousarray(x5.sum(axis=1)),
        "wts": np.ascontiguousarray(W.transpose(1, 0, 2, 3).reshape(KS, 80, 128)
                                    ).astype(np.float32),
        "bias": np.ascontiguousarray(b.reshape(128, 1)).astype(np.float32),
    }


def kernel(x, W, b):
    global LAST_EXEC_NS
    x = np.asarray(x, np.float32)
    W = np.asarray(W, np.float32)
    b = np.asarray(b, np.float32)

    if "mod" not in _NC_CACHE:
        _NC_CACHE["mod"] = _build_module()
    nc = _NC_CACHE["mod"]

    in_maps = [_prep_core_inputs(x, W, b, c) for c in range(8)]
    trace = os.environ.get("BASS_KERNEL_TRACE", "0") == "1"
    try:
        res = run_bass_kernel_spmd(nc, in_maps, core_ids=list(range(8)), trace=trace)
    except ModuleNotFoundError:
        res = run_bass_kernel_spmd(nc, in_maps, core_ids=list(range(8)), trace=False)
    LAST_EXEC_NS = res.exec_time_ns if res.exec_time_ns else res.mean_exec_time_ns
    if LAST_EXEC_NS is None:
        # No NTFF profiling hook under this axon client; fall back to the
        # instruction-level device-occupancy model (same cost tables CoreSim
        # uses), which is the best available per-core duration estimate.
        if "model_ns" not in _NC_CACHE:
            try:
                from concourse.timeline_sim import TimelineSim
                _NC_CACHE["model_ns"] = int(TimelineSim(nc, trace=False).simulate())
            except Exception:
                _NC_CACHE["model_ns"] = None
        LAST_EXEC_NS = _NC_CACHE["model_ns"]

    out = np.empty((4, 128, 128, NC, NA), np.float32)
    for c in range(8):
        bb, half = c // 2, c % 2
        r0 = 64 * half
        # y is [128 chan, PIX] channel-major (no on-chip transpose); pixel
        # index = row*WID + col within this core's 64-row slab.
        ych = res.results[c]["y"].reshape(128, ROWS, WID)
        out[bb, r0:r0 + 64] = ych.transpose(1, 2, 0).reshape(ROWS, WID, NC, NA)
    return out



# revision 20
# speedup vs baseline: 1.1347x; 1.0335x over previous
"""Trainium2 Bass kernel for ConvCapsuleLayer (SegCaps conv-capsule + dynamic routing).

Reference computation (per full input):
  x [B=4,H=128,W=128,IC=4,IA=16] -> conv 5x5 SAME over each of IC planes
  (IA in-channels -> NC*NA=128 out-channels) -> votes [IC,B,h,w,NC,NA]
  -> 3 iterations of dynamic routing (softmax over NC, squash over NA)
  -> activation [B,h,w,NC,NA]

Sharding: 8 cores = batch(4) x row-halves(2). Each core computes rows
[r0, r0+64) of one batch image: conv for all IC + full routing (routing is
per-pixel, so fully shard-local). Output gathered by concatenation.

Per-core layout: output channels (nc,na)=128 on SBUF partitions, pixels on
the free axis, processed in "quads" of 4 image rows (512 pixels); 4 quads
form a "super" so the small softmax/squash tensors pack 4 quads into one
[32,512]/[128,512] instruction. Full fp32 (routing amplifies input noise
~10x, so bf16 anywhere costs percent-level output error).

Conv: K=(dy,ia)=80 contraction, 5 matmuls (one per dx) accumulating in PSUM;
the input is staged host-side as 5 row-shifted copies (one per dy) so each
output row's receptive field is a contiguous AP slice. A 5th "summed over
IC" image gives routing iteration 1's uniform-route preactivation directly.
"""

import os

import numpy as np
import ml_dtypes

import concourse.bass as bass
import concourse.bacc as bacc
import concourse.mybir as mybir
import concourse.tile as tile
from concourse.bass_utils import run_bass_kernel_spmd

BF16 = mybir.dt.bfloat16
F32 = mybir.dt.float32
F32R = mybir.dt.float32r
AF = mybir.ActivationFunctionType
OP = mybir.AluOpType

IC, IA, NC, NA, KS = 4, 16, 8, 16, 5
ROWS = 64            # rows per core
WID = 128            # image width
PIX = ROWS * WID     # 8192 pixels per core
NSUPER = 4           # supers per core, 4 quads (16 rows) each

_NC_CACHE = {}
LAST_EXEC_NS = None


def _consts():
    """Stationary 0/1 matrices for partition-group reduce / broadcast."""
    # S16[j]: [128,32] reduce sq over na (groups of 16) into rows 8j..8j+8
    s16 = np.zeros((4, 128, 32), np.float32)
    for j in range(4):
        for m in range(32):
            if m // 8 == j:
                nc_i = m % 8
                s16[j, nc_i * 16:(nc_i + 1) * 16, m] = 1.0
    # S8x16[j]: [32,128] broadcast factor rows 8j+nc -> partitions (nc,na)
    s816 = np.zeros((4, 32, 128), np.float32)
    for j in range(4):
        for m in range(128):
            s816[j, 8 * j + m // 16, m] = 1.0
    # Bic[ic]: [32,128] broadcast route rows (8ic+nc) -> partitions (nc,na)
    bic = np.zeros((4, 32, 128), np.float32)
    for ic in range(4):
        for m in range(128):
            bic[ic, 8 * ic + m // 16, m] = 1.0
    # Sagr[j,ic]: [128,128] reduce G over na into logits rows 32j+8ic..+8
    sagr = np.zeros((16, 128, 128), np.float32)
    for j in range(4):
        for ic in range(4):
            for m in range(128):
                if m // 32 == j and (m % 32) // 8 == ic:
                    nc_i = m % 8
                    sagr[4 * j + ic, nc_i * 16:(nc_i + 1) * 16, m] = 1.0
    # D128: [128,128] block softmax denominator (sum+broadcast within 8-blocks)
    d128 = np.zeros((128, 128), np.float32)
    for k in range(128):
        for m in range(128):
            if k // 8 == m // 8:
                d128[k, m] = 1.0
    ident = np.eye(128, dtype=np.float32)
    return s16, s816, bic, sagr, d128, ident


def _build_module(mode=None):
    nc = bacc.Bacc(None, target_bir_lowering=False)
    x5_d = nc.declare_dram_parameter("x5", [80, IC, ROWS, WID + 4], F32, isOutput=False)
    xs5_d = nc.declare_dram_parameter("xs5", [80, ROWS, WID + 4], F32, isOutput=False)
    wts_d = nc.declare_dram_parameter("wts", [KS, 80, 128], F32, isOutput=False)
    bias_d = nc.declare_dram_parameter("bias", [128, 1], F32, isOutput=False)
    y_d = nc.declare_dram_parameter("y", [128, PIX], F32, isOutput=True)

    s16_np, s816_np, bic_np, sagr_np, d128_np, ident_np = _consts()
    s16_d = nc.inline_tensor(np.ascontiguousarray(s16_np.transpose(1, 0, 2)), "s16c")
    s816_d = nc.inline_tensor(np.ascontiguousarray(s816_np.transpose(1, 0, 2)), "s816c")
    bic_d = nc.inline_tensor(np.ascontiguousarray(bic_np.transpose(1, 0, 2)), "bicc")
    sagr_d = nc.inline_tensor(np.ascontiguousarray(sagr_np.transpose(1, 0, 2)), "sagrc")
    d128_d = nc.inline_tensor(d128_np, "d128c")
    ident_d = nc.inline_tensor(ident_np, "identc")

    with tile.TileContext(nc) as tc:
        _body(tc, x5_d, xs5_d, wts_d, bias_d, y_d,
              s16_d, s816_d, bic_d, sagr_d, d128_d, ident_d, mode=mode)
    _preload_act_table(nc)
    nc.compile()
    return nc


def _preload_act_table(nc):
    """Pre-place one LoadActFuncSet for the table containing ALL activation
    functions this kernel uses (exp, ln, identity, copy, square). Without it,
    the compile pass picks the first table per function (exp-only / ln-only)
    and thrashes 41 table reloads (1.3us each) onto the critical path."""
    set_id = 6  # natural_log_exp_and_others in act_info.json
    try:
        from concourse.hw_specs import get_activation_tables
        need = {AF.Exp, AF.Ln, AF.Identity, AF.Copy, AF.Square}
        for i, funcs in enumerate(get_activation_tables(nc.m.arch).values()):
            if need <= funcs:
                set_id = i
                break
    except Exception:
        pass
    entry = nc.main_func.blocks[0]
    atl = mybir.InstLoadActFuncSet(name=nc.get_next_instruction_name(),
                                   ins=[], outs=[])
    atl.engine = mybir.EngineType.Activation
    atl.act_func_set_id = set_id
    nc.register_instruction(atl)
    idx = entry.instructions.index(nc.gpsimd.preamble_end) + 1
    entry.instructions.insert(idx, atl)


def _body(tc, x5_d, xs5_d, wts_d, bias_d, y_d,
          s16_d, s816_d, bic_d, sagr_d, d128_d, ident_d, mode=None):
    if mode is None:
        mode = {"softmax", "preact", "squash", "agree", "out"}
    nc = tc.nc
    from contextlib import ExitStack
    ctx = ExitStack()
    with ctx:
        consts = ctx.enter_context(tc.tile_pool(name="consts", bufs=1))
        xpool = ctx.enter_context(tc.tile_pool(name="xin", bufs=4))
        vpool = ctx.enter_context(tc.tile_pool(name="votes", bufs=6))
        prepool = ctx.enter_context(tc.tile_pool(name="pre", bufs=6))
        apool = ctx.enter_context(tc.tile_pool(name="act", bufs=5))
        a3pool = ctx.enter_context(tc.tile_pool(name="act3", bufs=2))
        tmppool = ctx.enter_context(tc.tile_pool(name="tmp", bufs=2))
        gpool = ctx.enter_context(tc.tile_pool(name="gtmp", bufs=4))
        spool = ctx.enter_context(tc.tile_pool(name="smax", bufs=2))
        routepool = ctx.enter_context(tc.tile_pool(name="routes", bufs=4))
        smallpool = ctx.enter_context(tc.tile_pool(name="small", bufs=2))
        ps_conv = ctx.enter_context(tc.tile_pool(name="ps_conv", bufs=2, space="PSUM"))
        ps_rb = ctx.enter_context(tc.tile_pool(name="ps_rb", bufs=2, space="PSUM"))
        ps_logit = ctx.enter_context(tc.tile_pool(name="ps_logit", bufs=2, space="PSUM"))
        ps_misc = ctx.enter_context(tc.tile_pool(name="ps_misc", bufs=2, space="PSUM"))

        # ---- constants into SBUF
        w_sb = consts.tile([80, KS, 128], F32R)
        for dx in range(KS):
            nc.sync.dma_start(out=w_sb[:, dx, :], in_=wts_d[dx].bitcast(F32R))
        bias_sb = consts.tile([128, 1], F32)
        nc.sync.dma_start(out=bias_sb, in_=bias_d[:])
        s16_sb = consts.tile([128, 4, 32], F32R)
        nc.sync.dma_start(out=s16_sb, in_=s16_d[:].bitcast(F32R))
        s816_sb = consts.tile([32, 4, 128], F32R)
        nc.sync.dma_start(out=s816_sb, in_=s816_d[:].bitcast(F32R))
        bic_sb = consts.tile([32, 4, 128], F32R)
        nc.sync.dma_start(out=bic_sb, in_=bic_d[:].bitcast(F32R))
        sagr_sb = consts.tile([128, 16, 128], F32R)
        nc.sync.dma_start(out=sagr_sb, in_=sagr_d[:].bitcast(F32R))
        d128_sb = consts.tile([128, 128], F32R)
        nc.sync.dma_start(out=d128_sb, in_=d128_d[:].bitcast(F32R))
        id_sb = consts.tile([128, 128], F32)
        nc.sync.dma_start(out=id_sb, in_=ident_d[:])

        y_ap = y_d[:]

        def mm(out, lhsT, rhs, **kw):
            nc.tensor.matmul(out, lhsT=lhsT, rhs=rhs, **kw)

        xcs0 = None
        xss0 = None
        for s in range(NSUPER):
            # ---- stream this super's input rows (16) in two 8-row chunks
            if "nodma" in mode and xcs0 is not None:
                xcs, xss = xcs0, xss0
            else:
                xcs = []
                xss = []
                for h in range(4):
                    r = 16 * s + 4 * h
                    xc = xpool.tile([80, IC, 4, WID + 4], F32R, tag="xc")
                    nc.sync.dma_start(out=xc, in_=x5_d[:, :, r:r + 4, :].bitcast(F32R))
                    xcs.append(xc)
                    xsh = xpool.tile([80, 4, WID + 4], F32R, tag="xs")
                    nc.sync.dma_start(out=xsh, in_=xs5_d[:, r:r + 4, :].bitcast(F32R))
                    xss.append(xsh)
                xcs0, xss0 = xcs, xss

            # ---- conv: votes for 4 quads + uniform-route preactivation
            V = []
            P1 = []
            for j in range(4):
                vj = vpool.tile([128, IC, 512], F32, tag="vtile")
                for ic in range(IC):
                    pv = ps_conv.tile([128, 512], F32, tag="pconv")
                    for dx in range(KS):
                        mm(pv[:],
                           w_sb[:, dx, :],
                           xcs[j][:, ic, :, dx:dx + WID],
                           start=(dx == 0), stop=(dx == KS - 1))
                    nc.scalar.copy(out=vj[:, ic, :], in_=pv[:])
                V.append(vj)
                ps1 = ps_conv.tile([128, 512], F32, tag="pconv")
                for dx in range(KS):
                    mm(ps1[:], w_sb[:, dx, :],
                       xss[j][:, :, dx:dx + WID],
                       start=(dx == 0), stop=(dx == KS - 1))
                P1.append(ps1)

            logits = ps_logit.tile([128, 512], F32, tag="logits")
            if "agree" not in mode:
                nc.vector.memset(logits[:], 0.0)
            A = [None] * 4
            pre = [None] * 4
            for t in (1, 2, 3):
                if t == 1:
                    for j in range(4):
                        pj = prepool.tile([128, 512], F32, tag="pre")
                        nc.scalar.activation(pj, P1[j][:], AF.Identity,
                                             bias=bias_sb, scale=0.125)
                        pre[j] = pj
                elif "softmax" in mode:
                    expt = spool.tile([128, 512], F32R, tag="expt")
                    nc.scalar.activation(expt, logits[:], AF.Exp)
                    den = ps_misc.tile([128, 512], F32, tag="misc")
                    mm(den[:], d128_sb[:], expt[:], start=True, stop=True)
                    lnden = spool.tile([128, 512], F32, tag="lnden")
                    nc.scalar.activation(lnden, den[:], AF.Ln)
                    rden = spool.tile([128, 512], F32, tag="rden")
                    nc.scalar.activation(rden, lnden, AF.Exp, scale=-1.0)
                    routes = []
                    for j in range(4):
                        rj = routepool.tile([32, 512], F32R, tag="routes")
                        nc.vector.tensor_mul(rj,
                                             expt[32 * j:32 * j + 32, :],
                                             rden[32 * j:32 * j + 32, :])
                        routes.append(rj)
                    for j in range(4):
                        if "preact" not in mode:
                            if pre[j] is None:
                                dummy_pre = prepool.tile([128, 512], F32, tag="pre")
                                nc.vector.memset(dummy_pre, 0.0)
                                pre[j] = dummy_pre
                            continue
                        gs = []
                        for ic in range(IC):
                            rb = ps_rb.tile([128, 512], F32, tag="rb")
                            mm(rb[:], bic_sb[:, ic, :], routes[j][:],
                               start=True, stop=True)
                            g = gpool.tile([128, 512], F32, tag="g")
                            nc.vector.tensor_mul(g, V[j][:, ic, :], rb[:])
                            gs.append(g)
                        a01 = tmppool.tile([128, 512], F32, tag="s01")
                        nc.gpsimd.tensor_add(a01, gs[0], gs[1])
                        a23 = tmppool.tile([128, 512], F32, tag="s23")
                        nc.vector.tensor_add(a23, gs[2], gs[3])
                        acc = tmppool.tile([128, 512], F32, tag="vsum")
                        nc.vector.tensor_add(acc, a01, a23)
                        pj = prepool.tile([128, 512], F32, tag="pre")
                        nc.scalar.activation(pj, acc, AF.Identity, bias=bias_sb)
                        pre[j] = pj

                if "squash" not in mode:
                    for j in range(4):
                        if t < 3 and A[j] is None:
                            dummy_a = apool.tile([128, 512], F32, tag="atile")
                            nc.vector.memset(dummy_a, 0.0)
                            A[j] = dummy_a
                    continue
                # squash factor, packed over the 4 quads
                n2 = ps_misc.tile([32, 512], F32, tag="misc")
                for j in range(4):
                    sq = tmppool.tile([128, 512], F32R, tag="sq")
                    nc.scalar.activation(sq, pre[j], AF.Square)
                    mm(n2[:], s16_sb[:, j, :], sq[:], start=(j == 0), stop=(j == 3))
                ln1 = smallpool.tile([32, 512], F32, tag="ln1")
                nc.scalar.activation(ln1, n2[:], AF.Ln, bias=1.0)
                lnn = smallpool.tile([32, 512], F32, tag="lnn")
                nc.scalar.activation(lnn, n2[:], AF.Ln)
                tfac = smallpool.tile([32, 512], F32, tag="tfac")
                nc.vector.scalar_tensor_tensor(
                    out=tfac, in0=lnn, scalar=0.5, in1=ln1,
                    op0=OP.mult, op1=OP.subtract)
                fac = smallpool.tile([32, 512], F32R, tag="fac")
                nc.scalar.activation(fac, tfac, AF.Exp)

                for j in range(4):
                    fb = ps_rb.tile([128, 512], F32, tag="rb")
                    mm(fb[:], s816_sb[:, j, :], fac[:], start=True, stop=True)
                    if t < 3:
                        aj = apool.tile([128, 512], F32, tag="atile")
                        nc.vector.tensor_mul(aj, pre[j], fb[:])
                        A[j] = aj
                    elif "out" not in mode:
                        pass
                    else:
                        a3 = a3pool.tile([128, 512], F32, tag="a3")
                        nc.vector.tensor_mul(a3, pre[j], fb[:])
                        q = 4 * s + j
                        nc.sync.dma_start(out=y_ap[:, q * 512:(q + 1) * 512],
                                          in_=a3[:])

                if t < 3 and "agree" in mode:
                    for j in range(4):
                        for ic in range(IC):
                            g = gpool.tile([128, 512], F32R, tag="ag")
                            nc.gpsimd.tensor_mul(g, V[j][:, ic, :], A[j])
                            nc.tensor.matmul(
                                logits[:],
                                lhsT=sagr_sb[:, 4 * j + ic, :],
                                rhs=g[:],
                                start=(t == 1 and j == 0 and ic == 0),
                                stop=(t == 2 and j == 3 and ic == 3),
                                skip_group_check=True)


def _prep_core_inputs(x, W, b, core):
    bb, half = core // 2, core % 2
    r0 = 64 * half
    xb = x[bb]  # [H, W, IC, IA]
    xpad = np.zeros((IC, ROWS + 4, WID + 4, IA), np.float32)
    lo, hi = max(0, r0 - 2), min(128, r0 + 66)
    xpad[:, lo - (r0 - 2):hi - (r0 - 2), 2:WID + 2, :] = \
        xb[lo:hi].transpose(2, 0, 1, 3)
    x5 = np.empty((80, IC, ROWS, WID + 4), np.float32)
    for dy in range(KS):
        x5[dy * 16:(dy + 1) * 16] = xpad[:, dy:dy + ROWS, :, :].transpose(3, 0, 1, 2)
    return {
        "x5": x5,
        "xs5": np.ascontiguousarray(x5.sum(axis=1)),
        "wts": np.ascontiguousarray(W.transpose(1, 0, 2, 3).reshape(KS, 80, 128)
                                    ).astype(np.float32),
        "bias": np.ascontiguousarray(b.reshape(128, 1)).astype(np.float32),
    }


def kernel(x, W, b):
    global LAST_EXEC_NS
    x = np.asarray(x, np.float32)
    W = np.asarray(W, np.float32)
    b = np.asarray(b, np.float32)

    if "mod" not in _NC_CACHE:
        _NC_CACHE["mod"] = _build_module()
    nc = _NC_CACHE["mod"]

    in_maps = [_prep_core_inputs(x, W, b, c) for c in range(8)]
    trace = os.environ.get("BASS_KERNEL_TRACE", "0") == "1"
    try:
        res = run_bass_kernel_spmd(nc, in_maps, core_ids=list(range(8)), trace=trace)
    except ModuleNotFoundError:
        res = run_bass_kernel_spmd(nc, in_maps, core_ids=list(range(8)), trace=False)
    LAST_EXEC_NS = res.exec_time_ns if res.exec_time_ns else res.mean_exec_time_ns
    if LAST_EXEC_NS is None:
        # No NTFF profiling hook under this axon client; fall back to the
        # instruction-level device-occupancy model (same cost tables CoreSim
        # uses), which is the best available per-core duration estimate.
        if "model_ns" not in _NC_CACHE:
            try:
                from concourse.timeline_sim import TimelineSim
                _NC_CACHE["model_ns"] = int(TimelineSim(nc, trace=False).simulate())
            except Exception:
                _NC_CACHE["model_ns"] = None
        LAST_EXEC_NS = _NC_CACHE["model_ns"]

    out = np.empty((4, 128, 128, NC, NA), np.float32)
    for c in range(8):
        bb, half = c // 2, c % 2
        r0 = 64 * half
        # y is [128 chan, PIX] channel-major (no on-chip transpose); pixel
        # index = row*WID + col within this core's 64-row slab.
        ych = res.results[c]["y"].reshape(128, ROWS, WID)
        out[bb, r0:r0 + 64] = ych.transpose(1, 2, 0).reshape(ROWS, WID, NC, NA)
    return out



# revision 22
# speedup vs baseline: 1.1657x; 1.0273x over previous
"""Trainium2 Bass kernel for ConvCapsuleLayer (SegCaps conv-capsule + dynamic routing).

Reference computation (per full input):
  x [B=4,H=128,W=128,IC=4,IA=16] -> conv 5x5 SAME over each of IC planes
  (IA in-channels -> NC*NA=128 out-channels) -> votes [IC,B,h,w,NC,NA]
  -> 3 iterations of dynamic routing (softmax over NC, squash over NA)
  -> activation [B,h,w,NC,NA]

Sharding: 8 cores = batch(4) x row-halves(2). Each core computes rows
[r0, r0+64) of one batch image: conv for all IC + full routing (routing is
per-pixel, so fully shard-local). Output gathered by concatenation.

Per-core layout: output channels (nc,na)=128 on SBUF partitions, pixels on
the free axis, processed in "quads" of 4 image rows (512 pixels); 4 quads
form a "super" so the small softmax/squash tensors pack 4 quads into one
[32,512]/[128,512] instruction. Full fp32 (routing amplifies input noise
~10x, so bf16 anywhere costs percent-level output error).

Conv: K=(dy,ia)=80 contraction, 5 matmuls (one per dx) accumulating in PSUM;
the input is staged host-side as 5 row-shifted copies (one per dy) so each
output row's receptive field is a contiguous AP slice. A 5th "summed over
IC" image gives routing iteration 1's uniform-route preactivation directly.
"""

import os

import numpy as np
import ml_dtypes

import concourse.bass as bass
import concourse.bacc as bacc
import concourse.mybir as mybir
import concourse.tile as tile
from concourse.bass_utils import run_bass_kernel_spmd

BF16 = mybir.dt.bfloat16
F32 = mybir.dt.float32
F32R = mybir.dt.float32r
AF = mybir.ActivationFunctionType
OP = mybir.AluOpType

IC, IA, NC, NA, KS = 4, 16, 8, 16, 5
ROWS = 64            # rows per core
WID = 128            # image width
PIX = ROWS * WID     # 8192 pixels per core
NSUPER = 4           # supers per core, 4 quads (16 rows) each

_NC_CACHE = {}
LAST_EXEC_NS = None


def _consts():
    """Stationary 0/1 matrices for partition-group reduce / broadcast."""
    # S16[j]: [128,32] reduce sq over na (groups of 16) into rows 8j..8j+8
    s16 = np.zeros((4, 128, 32), np.float32)
    for j in range(4):
        for m in range(32):
            if m // 8 == j:
                nc_i = m % 8
                s16[j, nc_i * 16:(nc_i + 1) * 16, m] = 1.0
    # S8x16[j]: [32,128] broadcast factor rows 8j+nc -> partitions (nc,na)
    s816 = np.zeros((4, 32, 128), np.float32)
    for j in range(4):
        for m in range(128):
            s816[j, 8 * j + m // 16, m] = 1.0
    # Bic[ic]: [32,128] broadcast route rows (8ic+nc) -> partitions (nc,na)
    bic = np.zeros((4, 32, 128), np.float32)
    for ic in range(4):
        for m in range(128):
            bic[ic, 8 * ic + m // 16, m] = 1.0
    # Sagr[j,ic]: [128,128] reduce G over na into logits rows 32j+8ic..+8
    sagr = np.zeros((16, 128, 128), np.float32)
    for j in range(4):
        for ic in range(4):
            for m in range(128):
                if m // 32 == j and (m % 32) // 8 == ic:
                    nc_i = m % 8
                    sagr[4 * j + ic, nc_i * 16:(nc_i + 1) * 16, m] = 1.0
    # D128: [128,128] block softmax denominator (sum+broadcast within 8-blocks)
    d128 = np.zeros((128, 128), np.float32)
    for k in range(128):
        for m in range(128):
            if k // 8 == m // 8:
                d128[k, m] = 1.0
    ident = np.eye(128, dtype=np.float32)
    return s16, s816, bic, sagr, d128, ident


def _build_module(mode=None):
    nc = bacc.Bacc(None, target_bir_lowering=False)
    x5_d = nc.declare_dram_parameter("x5", [80, IC, ROWS, WID + 4], F32, isOutput=False)
    xs5_d = nc.declare_dram_parameter("xs5", [80, ROWS, WID + 4], F32, isOutput=False)
    wts_d = nc.declare_dram_parameter("wts", [KS, 80, 128], F32, isOutput=False)
    bias_d = nc.declare_dram_parameter("bias", [128, 1], F32, isOutput=False)
    y_d = nc.declare_dram_parameter("y", [128, PIX], F32, isOutput=True)

    s16_np, s816_np, bic_np, sagr_np, d128_np, ident_np = _consts()
    s16_d = nc.inline_tensor(np.ascontiguousarray(s16_np.transpose(1, 0, 2)), "s16c")
    s816_d = nc.inline_tensor(np.ascontiguousarray(s816_np.transpose(1, 0, 2)), "s816c")
    bic_d = nc.inline_tensor(np.ascontiguousarray(bic_np.transpose(1, 0, 2)), "bicc")
    sagr_d = nc.inline_tensor(np.ascontiguousarray(sagr_np.transpose(1, 0, 2)), "sagrc")
    d128_d = nc.inline_tensor(d128_np, "d128c")
    ident_d = nc.inline_tensor(ident_np, "identc")

    with tile.TileContext(nc) as tc:
        _body(tc, x5_d, xs5_d, wts_d, bias_d, y_d,
              s16_d, s816_d, bic_d, sagr_d, d128_d, ident_d, mode=mode)
    _preload_act_table(nc)
    nc.compile()
    return nc


def _preload_act_table(nc):
    """Pre-place one LoadActFuncSet for the table containing ALL activation
    functions this kernel uses (exp, ln, identity, copy, square). Without it,
    the compile pass picks the first table per function (exp-only / ln-only)
    and thrashes 41 table reloads (1.3us each) onto the critical path."""
    set_id = 6  # natural_log_exp_and_others in act_info.json
    try:
        from concourse.hw_specs import get_activation_tables
        need = {AF.Exp, AF.Ln, AF.Identity, AF.Copy, AF.Square}
        for i, funcs in enumerate(get_activation_tables(nc.m.arch).values()):
            if need <= funcs:
                set_id = i
                break
    except Exception:
        pass
    entry = nc.main_func.blocks[0]
    atl = mybir.InstLoadActFuncSet(name=nc.get_next_instruction_name(),
                                   ins=[], outs=[])
    atl.engine = mybir.EngineType.Activation
    atl.act_func_set_id = set_id
    nc.register_instruction(atl)
    idx = entry.instructions.index(nc.gpsimd.preamble_end) + 1
    entry.instructions.insert(idx, atl)


def _body(tc, x5_d, xs5_d, wts_d, bias_d, y_d,
          s16_d, s816_d, bic_d, sagr_d, d128_d, ident_d, mode=None):
    if mode is None:
        mode = {"softmax", "preact", "squash", "agree", "out"}
    nc = tc.nc
    from contextlib import ExitStack
    ctx = ExitStack()
    with ctx:
        consts = ctx.enter_context(tc.tile_pool(name="consts", bufs=1))
        xpool = ctx.enter_context(tc.tile_pool(name="xin", bufs=4))
        vpool = ctx.enter_context(tc.tile_pool(name="votes", bufs=6))
        prepool = ctx.enter_context(tc.tile_pool(name="pre", bufs=5))
        apool = ctx.enter_context(tc.tile_pool(name="act", bufs=5))
        a3pool = ctx.enter_context(tc.tile_pool(name="act3", bufs=5))
        tmppool = ctx.enter_context(tc.tile_pool(name="tmp", bufs=2))
        gpool = ctx.enter_context(tc.tile_pool(name="gtmp", bufs=4))
        spool = ctx.enter_context(tc.tile_pool(name="smax", bufs=2))
        routepool = ctx.enter_context(tc.tile_pool(name="routes", bufs=3))
        smallpool = ctx.enter_context(tc.tile_pool(name="small", bufs=1))
        ps_conv = ctx.enter_context(tc.tile_pool(name="ps_conv", bufs=2, space="PSUM"))
        ps_rb = ctx.enter_context(tc.tile_pool(name="ps_rb", bufs=2, space="PSUM"))
        ps_logit = ctx.enter_context(tc.tile_pool(name="ps_logit", bufs=2, space="PSUM"))
        ps_misc = ctx.enter_context(tc.tile_pool(name="ps_misc", bufs=2, space="PSUM"))

        # ---- constants into SBUF
        w_sb = consts.tile([80, KS, 128], F32R)
        for dx in range(KS):
            nc.sync.dma_start(out=w_sb[:, dx, :], in_=wts_d[dx].bitcast(F32R))
        bias_sb = consts.tile([128, 1], F32)
        nc.sync.dma_start(out=bias_sb, in_=bias_d[:])
        s16_sb = consts.tile([128, 4, 32], F32R)
        nc.sync.dma_start(out=s16_sb, in_=s16_d[:].bitcast(F32R))
        s816_sb = consts.tile([32, 4, 128], F32R)
        nc.sync.dma_start(out=s816_sb, in_=s816_d[:].bitcast(F32R))
        bic_sb = consts.tile([32, 4, 128], F32R)
        nc.sync.dma_start(out=bic_sb, in_=bic_d[:].bitcast(F32R))
        sagr_sb = consts.tile([128, 16, 128], F32R)
        nc.sync.dma_start(out=sagr_sb, in_=sagr_d[:].bitcast(F32R))
        d128_sb = consts.tile([128, 128], F32R)
        nc.sync.dma_start(out=d128_sb, in_=d128_d[:].bitcast(F32R))
        id_sb = consts.tile([128, 128], F32)
        nc.sync.dma_start(out=id_sb, in_=ident_d[:])

        y_ap = y_d[:]

        def mm(out, lhsT, rhs, **kw):
            nc.tensor.matmul(out, lhsT=lhsT, rhs=rhs, **kw)

        xcs0 = None
        xss0 = None
        for s in range(NSUPER):
            # ---- stream this super's input rows (16) in two 8-row chunks
            if "nodma" in mode and xcs0 is not None:
                xcs, xss = xcs0, xss0
            else:
                xcs = []
                xss = []
                for h in range(4):
                    r = 16 * s + 4 * h
                    xc = xpool.tile([80, IC, 4, WID + 4], F32R, tag="xc")
                    nc.sync.dma_start(out=xc, in_=x5_d[:, :, r:r + 4, :].bitcast(F32R))
                    xcs.append(xc)
                    xsh = xpool.tile([80, 4, WID + 4], F32R, tag="xs")
                    nc.sync.dma_start(out=xsh, in_=xs5_d[:, r:r + 4, :].bitcast(F32R))
                    xss.append(xsh)
                xcs0, xss0 = xcs, xss

            # ---- conv: votes for 4 quads + uniform-route preactivation
            V = []
            P1 = []
            for j in range(4):
                vj = vpool.tile([128, IC, 512], F32, tag="vtile")
                for ic in range(IC):
                    pv = ps_conv.tile([128, 512], F32, tag="pconv")
                    for dx in range(KS):
                        mm(pv[:],
                           w_sb[:, dx, :],
                           xcs[j][:, ic, :, dx:dx + WID],
                           start=(dx == 0), stop=(dx == KS - 1))
                    nc.scalar.copy(out=vj[:, ic, :], in_=pv[:])
                V.append(vj)
                ps1 = ps_conv.tile([128, 512], F32, tag="pconv")
                for dx in range(KS):
                    mm(ps1[:], w_sb[:, dx, :],
                       xss[j][:, :, dx:dx + WID],
                       start=(dx == 0), stop=(dx == KS - 1))
                # copy to SBUF immediately: a live PSUM tile on the pconv tag
                # would stall the next super's conv on this super's routing.
                p1sb = a3pool.tile([128, 512], F32, tag="p1sb")
                nc.scalar.copy(out=p1sb, in_=ps1[:])
                P1.append(p1sb)

            logits = ps_logit.tile([128, 512], F32, tag="logits")
            if "agree" not in mode:
                nc.vector.memset(logits[:], 0.0)
            A = [None] * 4
            pre = [None] * 4
            for t in (1, 2, 3):
                if t == 1:
                    for j in range(4):
                        pj = prepool.tile([128, 512], F32, tag="pre")
                        nc.scalar.activation(pj, P1[j][:], AF.Identity,
                                             bias=bias_sb, scale=0.125)
                        pre[j] = pj
                elif "softmax" in mode:
                    expt = spool.tile([128, 512], F32R, tag="expt")
                    nc.scalar.activation(expt, logits[:], AF.Exp)
                    den = ps_misc.tile([128, 512], F32, tag="misc")
                    mm(den[:], d128_sb[:], expt[:], start=True, stop=True)
                    lnden = spool.tile([128, 512], F32, tag="lnden")
                    nc.scalar.activation(lnden, den[:], AF.Ln)
                    rden = spool.tile([128, 512], F32, tag="rden")
                    nc.scalar.activation(rden, lnden, AF.Exp, scale=-1.0)
                    routes = []
                    for j in range(4):
                        rj = routepool.tile([32, 512], F32R, tag="routes")
                        nc.vector.tensor_mul(rj,
                                             expt[32 * j:32 * j + 32, :],
                                             rden[32 * j:32 * j + 32, :])
                        routes.append(rj)
                    for j in range(4):
                        if "preact" not in mode:
                            if pre[j] is None:
                                dummy_pre = prepool.tile([128, 512], F32, tag="pre")
                                nc.vector.memset(dummy_pre, 0.0)
                                pre[j] = dummy_pre
                            continue
                        gs = []
                        for ic in range(IC):
                            rb = ps_rb.tile([128, 512], F32, tag="rb")
                            mm(rb[:], bic_sb[:, ic, :], routes[j][:],
                               start=True, stop=True)
                            g = gpool.tile([128, 512], F32, tag="g")
                            nc.vector.tensor_mul(g, V[j][:, ic, :], rb[:])
                            gs.append(g)
                        a01 = tmppool.tile([128, 512], F32, tag="s01")
                        nc.gpsimd.tensor_add(a01, gs[0], gs[1])
                        a23 = tmppool.tile([128, 512], F32, tag="s23")
                        nc.vector.tensor_add(a23, gs[2], gs[3])
                        acc = tmppool.tile([128, 512], F32, tag="vsum")
                        nc.vector.tensor_add(acc, a01, a23)
                        pj = prepool.tile([128, 512], F32, tag="pre")
                        nc.scalar.activation(pj, acc, AF.Identity, bias=bias_sb)
                        pre[j] = pj

                if "squash" not in mode:
                    for j in range(4):
                        if t < 3 and A[j] is None:
                            dummy_a = apool.tile([128, 512], F32, tag="atile")
                            nc.vector.memset(dummy_a, 0.0)
                            A[j] = dummy_a
                    continue
                # squash factor, packed over the 4 quads
                n2 = ps_misc.tile([32, 512], F32, tag="misc")
                for j in range(4):
                    sq = tmppool.tile([128, 512], F32R, tag="sq")
                    nc.scalar.activation(sq, pre[j], AF.Square)
                    mm(n2[:], s16_sb[:, j, :], sq[:], start=(j == 0), stop=(j == 3))
                ln1 = smallpool.tile([32, 512], F32, tag="ln1")
                nc.scalar.activation(ln1, n2[:], AF.Ln, bias=1.0)
                lnn = smallpool.tile([32, 512], F32, tag="lnn")
                nc.scalar.activation(lnn, n2[:], AF.Ln)
                tfac = smallpool.tile([32, 512], F32, tag="tfac")
                nc.vector.scalar_tensor_tensor(
                    out=tfac, in0=lnn, scalar=0.5, in1=ln1,
                    op0=OP.mult, op1=OP.subtract)
                fac = smallpool.tile([32, 512], F32R, tag="fac")
                nc.scalar.activation(fac, tfac, AF.Exp)

                for j in range(4):
                    fb = ps_rb.tile([128, 512], F32, tag="rb")
                    mm(fb[:], s816_sb[:, j, :], fac[:], start=True, stop=True)
                    if t < 3:
                        aj = apool.tile([128, 512], F32, tag="atile")
                        nc.vector.tensor_mul(aj, pre[j], fb[:])
                        A[j] = aj
                    elif "out" not in mode:
                        pass
                    else:
                        a3 = a3pool.tile([128, 512], F32, tag="a3")
                        nc.vector.tensor_mul(a3, pre[j], fb[:])
                        q = 4 * s + j
                        nc.sync.dma_start(out=y_ap[:, q * 512:(q + 1) * 512],
                                          in_=a3[:])

                if t < 3 and "agree" in mode:
                    for j in range(4):
                        for ic in range(IC):
                            g = gpool.tile([128, 512], F32R, tag="ag")
                            nc.gpsimd.tensor_mul(g, V[j][:, ic, :], A[j])
                            nc.tensor.matmul(
                                logits[:],
                                lhsT=sagr_sb[:, 4 * j + ic, :],
                                rhs=g[:],
                                start=(t == 1 and j == 0 and ic == 0),
                                stop=(t == 2 and j == 3 and ic == 3),
                                skip_group_check=True)


def _prep_core_inputs(x, W, b, core):
    bb, half = core // 2, core % 2
    r0 = 64 * half
    xb = x[bb]  # [H, W, IC, IA]
    xpad = np.zeros((IC, ROWS + 4, WID + 4, IA), np.float32)
    lo, hi = max(0, r0 - 2), min(128, r0 + 66)
    xpad[:, lo - (r0 - 2):hi - (r0 - 2), 2:WID + 2, :] = \
        xb[lo:hi].transpose(2, 0, 1, 3)
    x5 = np.empty((80, IC, ROWS, WID + 4), np.float32)
    for dy in range(KS):
        x5[dy * 16:(dy + 1) * 16] = xpad[:, dy:dy + ROWS, :, :].transpose(3, 0, 1, 2)
    return {
        "x5": x5,
        "xs5": np.ascontiguousarray(x5.sum(axis=1)),
        "wts": np.ascontiguousarray(W.transpose(1, 0, 2, 3).reshape(KS, 80, 128)
                                    ).astype(np.float32),
        "bias": np.ascontiguousarray(b.reshape(128, 1)).astype(np.float32),
    }


def kernel(x, W, b):
    global LAST_EXEC_NS
    x = np.asarray(x, np.float32)
    W = np.asarray(W, np.float32)
    b = np.asarray(b, np.float32)

    if "mod" not in _NC_CACHE:
        _NC_CACHE["mod"] = _build_module()
    nc = _NC_CACHE["mod"]

    in_maps = [_prep_core_inputs(x, W, b, c) for c in range(8)]
    trace = os.environ.get("BASS_KERNEL_TRACE", "0") == "1"
    try:
        res = run_bass_kernel_spmd(nc, in_maps, core_ids=list(range(8)), trace=trace)
    except ModuleNotFoundError:
        res = run_bass_kernel_spmd(nc, in_maps, core_ids=list(range(8)), trace=False)
    LAST_EXEC_NS = res.exec_time_ns if res.exec_time_ns else res.mean_exec_time_ns
    if LAST_EXEC_NS is None:
        # No NTFF profiling hook under this axon client; fall back to the
        # instruction-level device-occupancy model (same cost tables CoreSim
        # uses), which is the best available per-core duration estimate.
        if "model_ns" not in _NC_CACHE:
            try:
                from concourse.timeline_sim import TimelineSim
                _NC_CACHE["model_ns"] = int(TimelineSim(nc, trace=False).simulate())
            except Exception:
                _NC_CACHE["model_ns"] = None
        LAST_EXEC_NS = _NC_CACHE["model_ns"]

    out = np.empty((4, 128, 128, NC, NA), np.float32)
    for c in range(8):
        bb, half = c // 2, c % 2
        r0 = 64 * half
        # y is [128 chan, PIX] channel-major (no on-chip transpose); pixel
        # index = row*WID + col within this core's 64-row slab.
        ych = res.results[c]["y"].reshape(128, ROWS, WID)
        out[bb, r0:r0 + 64] = ych.transpose(1, 2, 0).reshape(ROWS, WID, NC, NA)
    return out

